# revision 7
# baseline (speedup 1.0000x reference)
"""Dice loss kernel for Trainium2, 8 NeuronCores.

Problem: pred/target of shape (64, 1, 512, 512) f32. Per-row (batch) sums
p_sum, t_sum, inter=sum(p*t) -> dice loss -> mean over batch.

Sharding: data parallel over batch. Each of the 8 cores gets 8 rows
(8 MiB pred + 8 MiB target). On-device we compute per-partition partial
sums only; the tiny cross-partition / cross-core reduction plus the dice
formula run on the host.

Per-core layout: the 8 rows (8 * 262144 floats) are viewed as 4 chunks of
[128 partitions x 4096], i.e. one chunk = 2 consecutive rows; row 2c lands
in partitions 0..63, row 2c+1 in partitions 64..127. Each chunk is a single
contiguous 2 MiB DMA.

Per chunk:
  - inter: DVE tensor_tensor_reduce (fused multiply + free-dim reduce)
  - p_sum: ACT activation(Copy, accum_out=...)
  - t_sum: alternates ACT / DVE to balance the two engines
Stats tile [128, 12] (3 stats x 4 chunks) is DMA'd out at the end.
"""

import json

import numpy as np

import concourse.bass as bass
import concourse.bass2jax as bass2jax
import concourse.mybir as mybir
import concourse.tile as tile
from concourse.bass_utils import (
    compile_bir_kernel as _orig_compile_bir_kernel,
    run_bass_kernel_spmd,
)

# --- Workaround for the walrus build in this container -----------------------
# The walrus_driver here encodes at most ONE sync-wait per instruction
# (setupSyncWait "Too many sync wait commands" / visitInstISA "ISA wrong
# length" otherwise). Tile attaches multiple waits to one instruction, so
# before compiling we hoist all but the last wait of each instruction into
# single-wait NoOps on the same engine, inserted immediately before it in the
# same basic block (per-engine program order is block order, so semantics are
# identical).

_MAX_WAITS = 1


def _split_excess_waits(bir_json):
    bir = json.loads(bir_json)
    changed = False
    for fn in bir.get("functions", []):
        for blk in fn.get("blocks", []):
            insts = blk.get("instructions")
            if not insts:
                continue
            new = []
            for ins in insts:
                si = ins.get("sync_info") or {}
                ow = si.get("on_wait") or []
                if len(ow) > _MAX_WAITS:
                    changed = True
                    keep = ow[-_MAX_WAITS:]
                    for k, w in enumerate(ow[: -_MAX_WAITS]):
                        new.append(
                            {
                                "name": f"{ins['name']}-waitsplit{k}",
                                "opcode": "NoOp",
                                "engine": ins["engine"],
                                "ins": [],
                                "outs": [],
                                "debug": ins.get("debug", 0),
                                "is_reset_sema": False,
                                "sync_info": {"on_wait": [w], "on_update": []},
                            }
                        )
                    si["on_wait"] = keep
                new.append(ins)
            blk["instructions"] = new
    if not changed:
        return bir_json
    return json.dumps(bir).encode()


def _patched_compile_bir_kernel(bir_json, tmpdir, neff_name="file.neff"):
    return _orig_compile_bir_kernel(_split_excess_waits(bir_json), tmpdir, neff_name)


bass2jax.compile_bir_kernel = _patched_compile_bir_kernel
# -----------------------------------------------------------------------------

B = 64                 # batch rows total
N = 512 * 512          # elements per row
N_CORES = 8
ROWS_PER_CORE = B // N_CORES          # 8
P = 128                               # SBUF partitions
CHUNKS = 4                            # chunks per core
ROWS_PER_CHUNK = ROWS_PER_CORE // CHUNKS  # 2
F = ROWS_PER_CHUNK * N // P           # 4096 free-dim elems per chunk
PARTS_PER_ROW = P // ROWS_PER_CHUNK   # 64
SMOOTH = 1.0

_CACHE = {}


def _build_module(repeat=1):
    nc = bass.Bass()
    pred_d = nc.dram_tensor(
        "pred", [CHUNKS, P, F], mybir.dt.float32, kind="ExternalInput"
    )
    targ_d = nc.dram_tensor(
        "target", [CHUNKS, P, F], mybir.dt.float32, kind="ExternalInput"
    )
    stats_d = nc.dram_tensor(
        "stats", [P, 3 * CHUNKS], mybir.dt.float32, kind="ExternalOutput"
    )

    with tile.TileContext(nc) as tc:
        with (
            tc.tile_pool(name="pin", bufs=3) as pin_pool,
            tc.tile_pool(name="tin", bufs=3) as tin_pool,
            tc.tile_pool(name="scr", bufs=2) as scr_pool,
            tc.tile_pool(name="stats", bufs=2) as stats_pool,
        ):
            dummy = stats_pool.tile([P, 1], mybir.dt.float32, tag="dummy")

            for rep in range(repeat):
              stats_t = stats_pool.tile(
                  [P, 3 * CHUNKS], mybir.dt.float32, tag="stats"
              )
              for c in range(CHUNKS):
                p_t = pin_pool.tile([P, F], mybir.dt.float32, tag="p")
                t_t = tin_pool.tile([P, F], mybir.dt.float32, tag="t")
                nc.sync.dma_start(out=p_t, in_=pred_d[c])
                nc.sync.dma_start(out=t_t, in_=targ_d[c])

                # inter = sum_f(p*t) per partition on DVE (mult, then reduce)
                scr = scr_pool.tile([P, F], mybir.dt.float32, tag="scr")
                nc.vector.tensor_mul(out=scr, in0=p_t, in1=t_t)
                nc.vector.tensor_reduce(
                    out=stats_t[:, 3 * c : 3 * c + 1],
                    in_=scr,
                    axis=mybir.AxisListType.X,
                    op=mybir.AluOpType.add,
                )
                # p_sum / t_sum on ACT via activation-accumulate; the
                # full-shape `out` is a stride-0 dummy (never read)
                nc.scalar.activation(
                    out=dummy.broadcast_to([P, F]),
                    in_=p_t,
                    func=mybir.ActivationFunctionType.Copy,
                    accum_out=stats_t[:, 3 * c + 1 : 3 * c + 2],
                )
                nc.scalar.activation(
                    out=dummy.broadcast_to([P, F]),
                    in_=t_t,
                    func=mybir.ActivationFunctionType.Copy,
                    accum_out=stats_t[:, 3 * c + 2 : 3 * c + 3],
                )

            nc.sync.dma_start(out=stats_d[:], in_=stats_t)
    return nc


def get_module(repeat=1):
    key = ("nc", repeat)
    if key not in _CACHE:
        _CACHE[key] = _build_module(repeat)
    return _CACHE[key]


def make_in_maps(pred, target):
    """Full (64,1,512,512) inputs -> list of 8 per-core input dicts."""
    pred = np.ascontiguousarray(pred, dtype=np.float32).reshape(B, N)
    target = np.ascontiguousarray(target, dtype=np.float32).reshape(B, N)
    in_maps = []
    for core in range(N_CORES):
        rows = slice(core * ROWS_PER_CORE, (core + 1) * ROWS_PER_CORE)
        in_maps.append(
            {
                "pred": pred[rows].reshape(CHUNKS, P, F),
                "target": target[rows].reshape(CHUNKS, P, F),
            }
        )
    return in_maps


def finish_from_stats(stats_list):
    """stats_list: 8 arrays [128, 12] -> final scalar loss (np.float32)."""
    inter = np.empty(B, dtype=np.float64)
    p_sum = np.empty(B, dtype=np.float64)
    t_sum = np.empty(B, dtype=np.float64)
    for core, stats in enumerate(stats_list):
        s = np.asarray(stats, dtype=np.float64)
        for c in range(CHUNKS):
            for half in range(ROWS_PER_CHUNK):
                row = core * ROWS_PER_CORE + c * ROWS_PER_CHUNK + half
                parts = slice(half * PARTS_PER_ROW, (half + 1) * PARTS_PER_ROW)
                inter[row] = s[parts, 3 * c + 0].sum()
                p_sum[row] = s[parts, 3 * c + 1].sum()
                t_sum[row] = s[parts, 3 * c + 2].sum()
    dice = (2.0 * inter + SMOOTH) / (p_sum + t_sum + SMOOTH)
    losses = np.where(t_sum == 0.0, p_sum / N, 1.0 - dice)
    return np.asarray(losses.mean(), dtype=np.float32)


def kernel(pred, target, _run_kwargs=None, _repeat=1):
    nc = get_module(_repeat)
    in_maps = make_in_maps(pred, target)
    kwargs = _run_kwargs or {}
    res = run_bass_kernel_spmd(nc, in_maps, core_ids=list(range(N_CORES)), **kwargs)
    out = finish_from_stats([res.results[c]["stats"] for c in range(N_CORES)])
    if _run_kwargs is not None:
        _CACHE["last_results"] = res
    return out


# revision 25
# speedup vs baseline: 99.8925x; 99.8925x over previous
"""Dice loss kernel for Trainium2, 8 NeuronCores.

Problem: pred/target of shape (64, 1, 512, 512) f32. Per-row (batch) sums
p_sum, t_sum, inter=sum(p*t) -> dice loss -> mean over batch.

Sharding: data parallel over batch. Each of the 8 cores gets 8 rows
(8 MiB pred + 8 MiB target). On-device we compute per-partition partial
sums only; the tiny cross-partition / cross-core reduction plus the dice
formula run on the host.

Per-core layout: the 8 rows (8 * 262144 floats) are viewed as 4 chunks of
[128 partitions x 4096], i.e. one chunk = 2 consecutive rows; row 2c lands
in partitions 0..63, row 2c+1 in partitions 64..127. Each chunk is a single
contiguous 2 MiB DMA.

Per chunk:
  - inter: DVE tensor_tensor_reduce (fused multiply + free-dim reduce)
  - p_sum: ACT activation(Copy, accum_out=...)
  - t_sum: alternates ACT / DVE to balance the two engines
Stats tile [128, 12] (3 stats x 4 chunks) is DMA'd out at the end.
"""

import json

import numpy as np

import concourse.bass as bass
import concourse.bass2jax as bass2jax
import concourse.mybir as mybir
import concourse.tile as tile
from concourse.bass_utils import (
    compile_bir_kernel as _orig_compile_bir_kernel,
    run_bass_kernel_spmd,
)

# --- Workaround for the walrus build in this container -----------------------
# The walrus_driver here encodes at most ONE sync-wait per instruction
# (setupSyncWait "Too many sync wait commands" / visitInstISA "ISA wrong
# length" otherwise). Tile attaches multiple waits to one instruction, so
# before compiling we hoist all but the last wait of each instruction into
# single-wait NoOps on the same engine, inserted immediately before it in the
# same basic block (per-engine program order is block order, so semantics are
# identical).

_MAX_WAITS = 1


def _split_excess_waits(bir_json):
    bir = json.loads(bir_json)
    changed = False
    for fn in bir.get("functions", []):
        for blk in fn.get("blocks", []):
            insts = blk.get("instructions")
            if not insts:
                continue
            new = []
            for ins in insts:
                si = ins.get("sync_info") or {}
                ow = si.get("on_wait") or []
                if len(ow) > _MAX_WAITS:
                    changed = True
                    keep = ow[-_MAX_WAITS:]
                    for k, w in enumerate(ow[: -_MAX_WAITS]):
                        new.append(
                            {
                                "name": f"{ins['name']}-waitsplit{k}",
                                "opcode": "NoOp",
                                "engine": ins["engine"],
                                "ins": [],
                                "outs": [],
                                "debug": ins.get("debug", 0),
                                "is_reset_sema": False,
                                "sync_info": {"on_wait": [w], "on_update": []},
                            }
                        )
                    si["on_wait"] = keep
                new.append(ins)
            blk["instructions"] = new
    if not changed:
        return bir_json
    return json.dumps(bir).encode()


def _patched_compile_bir_kernel(bir_json, tmpdir, neff_name="file.neff"):
    neff_path = _orig_compile_bir_kernel(
        _split_excess_waits(bir_json), tmpdir, neff_name
    )
    try:
        import shutil
        import tempfile

        keep = tempfile.mkdtemp(prefix="kernel_neff_")
        kept = keep + "/" + neff_name
        shutil.copy(neff_path, kept)
        _CACHE["last_neff"] = kept
    except Exception:
        pass
    return neff_path


bass2jax.compile_bir_kernel = _patched_compile_bir_kernel
# -----------------------------------------------------------------------------

B = 64                 # batch rows total
N = 512 * 512          # elements per row
N_CORES = 8
ROWS_PER_CORE = B // N_CORES          # 8
P = 128                               # SBUF partitions
CHUNKS = 4                            # chunks per core (legacy tile build)
ROWS_PER_CHUNK = ROWS_PER_CORE // CHUNKS  # 2
F = ROWS_PER_CHUNK * N // P           # 4096 free-dim elems per chunk
PARTS_PER_ROW = P // ROWS_PER_CHUNK   # 64
SMOOTH = 1.0
SCOLS = 3                             # stats columns per chunk: inter, p_sum, t_sum

# Raw build: variable chunks, big first so the after-last-byte tail (serial
# DVE mul+reduce of the final chunk) is small. (flat_offset, F, kind, row):
#   pair   [128,4096]: rows row..row+1 via partition halves (64 each)
#   single [128,2048]: one row across all 128 partitions
#   half   [128,1024]: half of `row`; both halves sum into the row
CHUNK_LAYOUT = [
    (0 * N, 4096, "pair", 0),
    (2 * N, 4096, "pair", 2),
    (4 * N, 4096, "pair", 4),
    (6 * N, 2048, "single", 6),
    (7 * N, 1024, "half", 7),
    (7 * N + 128 * 1024, 1024, "half", 7),
]
NCHUNKS = len(CHUNK_LAYOUT)

_CACHE = {}


def _build_module_raw(repeat=1, clears=True):
    """Raw-bass pipeline: explicit semaphores, no Tile scheduler.

    Avoids Tile's ~15 us sem-init preamble and ~10 us EVSEM butterfly tail.
    All 4 p-loads issue immediately from SP (HWDGE ring 1), all 4 t-loads
    from ACT (HWDGE ring 2); DVE does mult + full pt-reduce, ACT does the
    p/t sums via activation-accumulate. 4-deep buffering -> no slot-reuse
    waits for loads; scr double-buffer is safe by DVE program order.
    """
    from contextlib import ExitStack

    # clears=True restores all sems to 0 at the end so the NEFF can be
    # re-executed; safe without a barrier because SP's final waits prove every
    # consumer already passed its last wait. The sim-only race detector can't
    # see that transitivity, so it is disabled for this build (a clears=False
    # build is validated with the detector on in test_sim.py).
    nc = bass.Bass(detect_race_conditions=not clears)
    pred_d = nc.dram_tensor(
        "pred", [ROWS_PER_CORE * N], mybir.dt.float32, kind="ExternalInput"
    )
    targ_d = nc.dram_tensor(
        "target", [ROWS_PER_CORE * N], mybir.dt.float32, kind="ExternalInput"
    )
    stats_d = nc.dram_tensor(
        "stats", [P, SCOLS * NCHUNKS], mybir.dt.float32, kind="ExternalOutput"
    )

    def chunk_ap(dram, i):
        off, fc, _, _ = CHUNK_LAYOUT[i]
        return dram[off : off + P * fc].rearrange("(p f) -> p f", f=fc)

    with ExitStack() as ctx:
        p_bufs = [
            ctx.enter_context(
                nc.sbuf_tensor(f"pbuf{i}", [P, CHUNK_LAYOUT[i][1]], mybir.dt.float32)
            )
            for i in range(NCHUNKS)
        ]
        t_bufs = [
            ctx.enter_context(
                nc.sbuf_tensor(f"tbuf{i}", [P, CHUNK_LAYOUT[i][1]], mybir.dt.float32)
            )
            for i in range(NCHUNKS)
        ]
        scr = [
            ctx.enter_context(nc.sbuf_tensor(f"scr{i}", [P, 4096], mybir.dt.float32))
            for i in range(2)
        ]
        stats = ctx.enter_context(
            nc.sbuf_tensor("statsbuf", [P, SCOLS * NCHUNKS], mybir.dt.float32)
        )
        dummy = ctx.enter_context(
            nc.sbuf_tensor("dummybuf", [P, 1], mybir.dt.float32)
        )
        sp = [
            ctx.enter_context(nc.semaphore(f"sem_p{i}")) for i in range(NCHUNKS)
        ]
        st = [
            ctx.enter_context(nc.semaphore(f"sem_t{i}")) for i in range(NCHUNKS)
        ]
        sv = ctx.enter_context(nc.semaphore("sem_v"))
        sa = ctx.enter_context(nc.semaphore("sem_a"))
        so = ctx.enter_context(nc.semaphore("sem_o"))
        block = ctx.enter_context(nc.Block())

        n_dve = 2 * NCHUNKS * repeat
        n_act = 2 * NCHUNKS * repeat

        @block.sync
        def _(sync):
            for rep in range(repeat):
                for c in range(NCHUNKS):
                    if rep > 0:
                        k_prev = (rep - 1) * NCHUNKS + c
                        sync.wait_ge(sv, 2 * k_prev + 1)
                        sync.wait_ge(sa, 2 * k_prev + 1)
                    sync.dma_start(
                        out=p_bufs[c][:], in_=chunk_ap(pred_d, c)
                    ).then_inc(sp[c], 16)
            sync.wait_ge(sv, n_dve)
            sync.wait_ge(sa, n_act)
            sync.dma_start(out=stats_d[:], in_=stats[:]).then_inc(so, 16)
            sync.wait_ge(so, 16)
            # sp/st are provably at 16*repeat here: sv/sa at full count means
            # every consumer's data-arrival wait passed, which requires the
            # DMA-completion increments to have landed.
            if clears:
                for sem in [*sp, *st, sv, sa, so]:
                    sync.sem_clear(sem)

        @block.scalar
        def _(scalar):
            for rep in range(repeat):
                for c in range(NCHUNKS):
                    if rep > 0:
                        k_prev = (rep - 1) * NCHUNKS + c
                        scalar.wait_ge(sv, 2 * k_prev + 1)
                        scalar.wait_ge(sa, 2 * k_prev + 2)
                    scalar.dma_start(
                        out=t_bufs[c][:], in_=chunk_ap(targ_d, c)
                    ).then_inc(st[c], 16)
            for rep in range(repeat):
                for c in range(NCHUNKS):
                    k = rep * NCHUNKS + c
                    fc = CHUNK_LAYOUT[c][1]
                    scalar.wait_ge(sp[c], 16 * (rep + 1))
                    if k > 0:
                        scalar.wait_ge(sa, 2 * k)
                    nc.scalar.activation(
                        out=dummy[:].broadcast_to([P, fc]),
                        in_=p_bufs[c][:],
                        func=mybir.ActivationFunctionType.Copy,
                        accum_out=stats[:, SCOLS * c + 1 : SCOLS * c + 2],
                    ).then_inc(sa, 1)
                    scalar.wait_ge(st[c], 16 * (rep + 1))
                    scalar.wait_ge(sa, 2 * k + 1)
                    nc.scalar.activation(
                        out=dummy[:].broadcast_to([P, fc]),
                        in_=t_bufs[c][:],
                        func=mybir.ActivationFunctionType.Copy,
                        accum_out=stats[:, SCOLS * c + 2 : SCOLS * c + 3],
                    ).then_inc(sa, 1)

        @block.vector
        def _(vector):
            for rep in range(repeat):
                for c in range(NCHUNKS):
                    k = rep * NCHUNKS + c
                    fc = CHUNK_LAYOUT[c][1]
                    vector.wait_ge(sp[c], 16 * (rep + 1))
                    vector.wait_ge(st[c], 16 * (rep + 1))
                    nc.vector.tensor_mul(
                        out=scr[k % 2][:, :fc], in0=p_bufs[c][:], in1=t_bufs[c][:]
                    ).then_inc(sv, 1)
                    vector.wait_ge(sv, 2 * k + 1)
                    nc.vector.tensor_reduce(
                        out=stats[:, SCOLS * c : SCOLS * c + 1],
                        in_=scr[k % 2][:, :fc],
                        axis=mybir.AxisListType.X,
                        op=mybir.AluOpType.add,
                    ).then_inc(sv, 1)

    return nc


def _build_module(repeat=1):
    nc = bass.Bass()
    pred_d = nc.dram_tensor(
        "pred", [CHUNKS, P, F], mybir.dt.float32, kind="ExternalInput"
    )
    targ_d = nc.dram_tensor(
        "target", [CHUNKS, P, F], mybir.dt.float32, kind="ExternalInput"
    )
    stats_d = nc.dram_tensor(
        "stats", [P, SCOLS * CHUNKS], mybir.dt.float32, kind="ExternalOutput"
    )

    with tile.TileContext(nc) as tc:
        with (
            tc.tile_pool(name="pin", bufs=3) as pin_pool,
            tc.tile_pool(name="tin", bufs=3) as tin_pool,
            tc.tile_pool(name="scr", bufs=2) as scr_pool,
            tc.tile_pool(name="stats", bufs=2) as stats_pool,
        ):
            dummy = stats_pool.tile([P, 1], mybir.dt.float32, tag="dummy")

            for rep in range(repeat):
              stats_t = stats_pool.tile(
                  [P, SCOLS * CHUNKS], mybir.dt.float32, tag="stats"
              )
              for c in range(CHUNKS):
                p_t = pin_pool.tile([P, F], mybir.dt.float32, tag="p")
                t_t = tin_pool.tile([P, F], mybir.dt.float32, tag="t")
                # two HWDGE rings: p via SP (nc.sync), t via ACT (nc.scalar)
                nc.sync.dma_start(out=p_t, in_=pred_d[c])
                nc.scalar.dma_start(out=t_t, in_=targ_d[c])

                # inter = sum_f(p*t): DVE does the mult and reduces the
                # first XSPLIT columns; ACT accumulates the rest of scr
                scr = scr_pool.tile([P, F], mybir.dt.float32, tag="scr")
                nc.vector.tensor_mul(out=scr, in0=p_t, in1=t_t)
                nc.vector.tensor_reduce(
                    out=stats_t[:, SCOLS * c : SCOLS * c + 1],
                    in_=scr,
                    axis=mybir.AxisListType.X,
                    op=mybir.AluOpType.add,
                )
                # p_sum / t_sum on ACT via activation-accumulate; the
                # full-shape `out` is a stride-0 dummy (never read)
                nc.scalar.activation(
                    out=dummy.broadcast_to([P, F]),
                    in_=p_t,
                    func=mybir.ActivationFunctionType.Copy,
                    accum_out=stats_t[:, SCOLS * c + 1 : SCOLS * c + 2],
                )
                nc.scalar.activation(
                    out=dummy.broadcast_to([P, F]),
                    in_=t_t,
                    func=mybir.ActivationFunctionType.Copy,
                    accum_out=stats_t[:, SCOLS * c + 2 : SCOLS * c + 3],
                )

            nc.sync.dma_start(out=stats_d[:], in_=stats_t)
    return nc


RAW = True


def get_module(repeat=1, clears=True):
    key = ("nc", repeat, RAW, clears)
    if key not in _CACHE:
        if RAW:
            _CACHE[key] = _build_module_raw(repeat, clears=clears)
        else:
            _CACHE[key] = _build_module(repeat)
    return _CACHE[key]


def make_in_maps(pred, target):
    """Full (64,1,512,512) inputs -> list of 8 per-core input dicts."""
    pred = np.ascontiguousarray(pred, dtype=np.float32).reshape(B, N)
    target = np.ascontiguousarray(target, dtype=np.float32).reshape(B, N)
    in_maps = []
    for core in range(N_CORES):
        rows = slice(core * ROWS_PER_CORE, (core + 1) * ROWS_PER_CORE)
        if RAW:
            in_maps.append(
                {
                    "pred": np.ascontiguousarray(pred[rows]).reshape(-1),
                    "target": np.ascontiguousarray(target[rows]).reshape(-1),
                }
            )
        else:
            in_maps.append(
                {
                    "pred": pred[rows].reshape(CHUNKS, P, F),
                    "target": target[rows].reshape(CHUNKS, P, F),
                }
            )
    return in_maps


def core_stats_to_sums(stats):
    """[P, SCOLS*NCHUNKS] -> (inter[8], p_sum[8], t_sum[8]) for one core."""
    s = np.asarray(stats, dtype=np.float64)
    inter = np.zeros(ROWS_PER_CORE)
    p_sum = np.zeros(ROWS_PER_CORE)
    t_sum = np.zeros(ROWS_PER_CORE)
    for c, (_, fc, kind, row) in enumerate(CHUNK_LAYOUT):
        cols = [SCOLS * c + 0, SCOLS * c + 1, SCOLS * c + 2]
        if kind == "pair":
            for half, r in ((0, row), (1, row + 1)):
                parts = slice(half * 64, (half + 1) * 64)
                inter[r] += s[parts, cols[0]].sum()
                p_sum[r] += s[parts, cols[1]].sum()
                t_sum[r] += s[parts, cols[2]].sum()
        else:  # single or half: whole chunk belongs to `row`
            inter[row] += s[:, cols[0]].sum()
            p_sum[row] += s[:, cols[1]].sum()
            t_sum[row] += s[:, cols[2]].sum()
    return inter, p_sum, t_sum


def finish_from_stats(stats_list):
    """stats_list: 8 arrays [128, SCOLS*NCHUNKS] -> final scalar loss."""
    inter = np.empty(B, dtype=np.float64)
    p_sum = np.empty(B, dtype=np.float64)
    t_sum = np.empty(B, dtype=np.float64)
    for core, stats in enumerate(stats_list):
        i, pz, tz = core_stats_to_sums(stats)
        rows = slice(core * ROWS_PER_CORE, (core + 1) * ROWS_PER_CORE)
        inter[rows] = i
        p_sum[rows] = pz
        t_sum[rows] = tz
    dice = (2.0 * inter + SMOOTH) / (p_sum + t_sum + SMOOTH)
    losses = np.where(t_sum == 0.0, p_sum / N, 1.0 - dice)
    return np.asarray(losses.mean(), dtype=np.float32)


def kernel(pred, target, _run_kwargs=None, _repeat=1):
    nc = get_module(_repeat)
    in_maps = make_in_maps(pred, target)
    kwargs = _run_kwargs or {}
    # The axon-tunneled devices intermittently report
    # NRT_EXEC_UNIT_UNRECOVERABLE on a first execution and recover on the
    # next attempt; retry a couple of times before giving up.
    last_exc = None
    for attempt in range(3):
        try:
            res = run_bass_kernel_spmd(
                nc, in_maps, core_ids=list(range(N_CORES)), **kwargs
            )
            break
        except Exception as exc:  # transient device failures included
            last_exc = exc
            import time as _time

            _time.sleep(5)
    else:
        raise last_exc
    out = finish_from_stats([res.results[c]["stats"] for c in range(N_CORES)])
    if _run_kwargs is not None:
        _CACHE["last_results"] = res
    return out
